# revision 12
# baseline (speedup 1.0000x reference)
"""AltConv via Winograd F(8,4) in fp16 on 8 TRN2 NeuronCores.

out[s] = sum_{i=0..3} K_i x[s-i].  Outputs in blocks of M=8 from NJ=11
Winograd-channel matmuls instead of 32 (2.9x fewer PE cycles than direct):

  w_l(u) = x[8u-3+l], l=0..10
  x~_j = alpha_j (BT w)_j   (host f64)    K~_j = beta_j (G Krev)_j (host f64)
  P_j  = x~_j @ K~_j        (device TensorE fp16, f32 PSUM accum over D)
  out[8u+t] = sum_j A[t,j]/(alpha_j beta_j) P_j(u)

Points {0, +-1, +-a, +-b, +-c, d, inf} (numerically optimized); per-channel
scales keep every fp16 plane in range; zero/inf channels scaled so their
combine coefficient is exactly 1 (plain adds).

Sharding: data-parallel over (batch, seq-half) -> 8 shards of T=4096 tokens,
U=512 output blocks each (one PSUM bank per P plane, rotating pool of 6).
Combine split across engines: ScalarE drains planes to fp16 + scales the
single-point channel per t; DVE does S/D pairs + 3 stt per output t;
GpSimd (parallel SBUF port) adds the pre-scaled single/zero/inf terms and
issues fp16 output stores.  First two feature blocks are channel-interleaved
so the PE keeps pace with the initial x~ stream.
"""

import numpy as np

B, S, D, F, R = 4, 8192, 1024, 1024, 4
N_CORES = 8
T = S // 2            # tokens per core
M = 8                 # outputs per Winograd block
NJ = M + R - 1        # 11 Winograd channels
KD = D // 128
FB = F // 128
U = T // M            # 512 blocks per core (exact)
RAMP = 2              # feature blocks interleaved during x~ stream-in

# optimized points: pairs (1, a, b, c), single d, zero, inf
PAIR_VALS = [1.0, 0.3744, 0.7256, 1.6749]
SINGLES = [4.0762]
NPAIR = len(PAIR_VALS)
J_SINGLE = 2 * NPAIR          # 8
J_ZERO = NJ - 2               # 9
J_INF = NJ - 1                # 10
J_ORDER = [J_SINGLE] + list(range(2 * NPAIR)) + [J_ZERO, J_INF]
_CACHE = {}


def _mats():
    pts = []
    for p in PAIR_VALS:
        pts += [p, -p]
    pts += list(SINGLES) + [0.0]
    n = NJ
    G = np.zeros((n, R))
    for j, p in enumerate(pts):
        G[j] = [p ** e for e in range(R)]
    G[-1, R - 1] = 1.0
    V = np.zeros((n, n))
    for j, p in enumerate(pts):
        V[j] = [p ** e for e in range(n)]
    V[-1, -1] = 1.0
    BT = np.linalg.inv(V).T
    A = np.zeros((M, n))
    for j, p in enumerate(pts):
        A[:, j] = [p ** t for t in range(M)]
    A[M - 1, n - 1] = 1.0
    alpha = 1.0 / np.linalg.norm(BT, axis=1)
    beta = 64.0 / np.linalg.norm(G, axis=1)
    for i in range(NPAIR):
        alpha[2 * i] = alpha[2 * i + 1] = min(alpha[2 * i], alpha[2 * i + 1])
        beta[2 * i] = beta[2 * i + 1] = min(beta[2 * i], beta[2 * i + 1])
    # chain base (pair 1) and the zero/inf add-terms need coefficient 1
    beta[0] = beta[1] = 1.0 / alpha[0]
    beta[J_ZERO] = 1.0 / alpha[J_ZERO]
    beta[J_INF] = 1.0 / alpha[J_INF]
    Ap = A / (alpha * beta)[None, :]
    assert abs(Ap[0, J_ZERO] - 1.0) < 1e-12
    assert abs(Ap[M - 1, J_INF] - 1.0) < 1e-12
    return G, BT, A, Ap, alpha, beta


def _build():
    if "nc" in _CACHE:
        return _CACHE["nc"]
    import concourse.tile as tile
    from concourse import bacc, mybir

    _, _, _, Ap, _, _ = _mats()
    nc = bacc.Bacc("TRN2", target_bir_lowering=False, debug=False,
                   num_devices=N_CORES)
    f16 = mybir.dt.float16
    f32 = mybir.dt.float32
    mult = mybir.AluOpType.mult
    add = mybir.AluOpType.add

    # channel storage on DRAM is permuted to processing order J_ORDER;
    # position pj holds math-channel J_ORDER[pj]
    NA = 6                       # channels in the first kt group tile
    xt_d = nc.dram_tensor("xt", [128, NJ, KD, U], f16, kind="ExternalInput")
    kt_d = nc.dram_tensor("kt", [FB, 128, NJ, KD, 128], f16,
                          kind="ExternalInput")
    out_d = nc.dram_tensor("outT", [FB, 128, M, U], f16,
                           kind="ExternalOutput")

    with tile.TileContext(nc) as tc:
        with (
            tc.tile_pool(name="xpool", bufs=1) as xpool,
            tc.tile_pool(name="kpool", bufs=1) as kpool,
            tc.tile_pool(name="psum", bufs=1, space="PSUM") as ppool,
            tc.tile_pool(name="stage", bufs=1) as spool,
        ):
            xt = xpool.tile([128, NJ, KD, U], f16)
            state = {}

            def new_kt(fb):
                # two channel-group tiles so buffer recycling (and thus the
                # next block's prefetch) happens at half-block granularity
                ka = kpool.tile([128, NA, KD, 128], f16, name=f"ktA{fb}",
                                tag="ktA", bufs=2)
                kb = kpool.tile([128, NJ - NA, KD, 128], f16,
                                name=f"ktB{fb}", tag="ktB", bufs=2)
                return (ka, kb)

            def emit_channel(fb, pj, kt):
                j = J_ORDER[pj]
                ktile = kt[0] if pj < NA else kt[1]
                lj = pj if pj < NA else pj - NA
                st, pc, sd, tmpd, tmpc = state[fb]
                P = ppool.tile([128, U], f32, tag="P", bufs=6,
                               name=f"P{fb}_{j}")
                for kd in range(KD):
                    nc.tensor.matmul(
                        P, ktile[:, lj, kd, :], xt[:, pj, kd, :],
                        start=(kd == 0), stop=(kd == KD - 1),
                    )
                if j == J_SINGLE:
                    # ScalarE scales the single-point plane per t, from PSUM
                    for t in range(M):
                        nc.scalar.mul(tmpd[:, t, :], P,
                                      float(Ap[t, J_SINGLE]))
                else:
                    tag = "pcp" if j < 2 * NPAIR else f"pc{j}"
                    bufs = 4 if j < 2 * NPAIR else 2
                    c = spool.tile([128, U], f16, tag=tag, bufs=bufs,
                                   name=f"pc{fb}_{j}")
                    nc.scalar.copy(c, P)
                    pc[j] = c
                    if j < 2 * NPAIR and j % 2 == 1:
                        i = j // 2
                        s_ = spool.tile([128, U], f16, tag=f"sd{i}S",
                                        bufs=2, name=f"S{fb}_{i}")
                        d_ = spool.tile([128, U], f16, tag=f"sd{i}D",
                                        bufs=2, name=f"D{fb}_{i}")
                        nc.vector.tensor_add(s_, pc[j - 1], pc[j])
                        nc.vector.tensor_sub(d_, pc[j - 1], pc[j])
                        sd[(i, 0)] = s_
                        sd[(i, 1)] = d_
                        if i == NPAIR - 1:
                            # ScalarE pre-scales the c-pair term per t
                            for t in range(M):
                                nc.scalar.mul(tmpc[:, t, :], sd[(i, t % 2)],
                                              float(Ap[t, 2 * i]))

            def open_fb(fb):
                st = spool.tile([128, M, U], f16, tag="st", bufs=2,
                                name=f"st{fb}")
                tmpd = spool.tile([128, M, U], f16, tag="tmpd", bufs=2,
                                  name=f"tmpd{fb}")
                tmpc = spool.tile([128, M, U], f16, tag="tmpc", bufs=2,
                                  name=f"tmpc{fb}")
                state[fb] = (st, {}, {}, tmpd, tmpc)

            def emit_combine(fb):
                st, pc, sd, tmpd, tmpc = state[fb]
                for t in range(M):
                    acc = st[:, t, :]
                    par = t % 2
                    nc.vector.scalar_tensor_tensor(
                        acc, sd[(1, par)], float(Ap[t, 2]), sd[(0, par)],
                        mult, add)
                    nc.vector.scalar_tensor_tensor(
                        acc, sd[(2, par)], float(Ap[t, 4]), acc,
                        mult, add)
                    # unscaled accumulates ride the SDMA CCE adders
                    nc.gpsimd.dma_start(acc, tmpc[:, t, :], accum_op=add)
                    nc.gpsimd.dma_start(acc, tmpd[:, t, :], accum_op=add)
                    if t == 0:
                        nc.gpsimd.dma_start(acc, pc[J_ZERO], accum_op=add)
                    if t == M - 1:
                        nc.gpsimd.dma_start(acc, pc[J_INF], accum_op=add)
                nc.gpsimd.dma_start(out_d[fb, :, :, :], st[:, :, :])

            def dma_kt(fb, kt, pj_lo, pj_hi):
                for pj in range(pj_lo, pj_hi):
                    ktile = kt[0] if pj < NA else kt[1]
                    lj = pj if pj < NA else pj - NA
                    nc.sync.dma_start(ktile[:, lj, :, :],
                                      kt_d[fb, :, pj, :, :])

            # ---- ramp: first RAMP feature blocks channel-interleaved ----
            kts = [new_kt(fb) for fb in range(RAMP)]
            for fb in range(RAMP):
                open_fb(fb)
            for pj in range(NJ):
                for fb in range(RAMP):
                    dma_kt(fb, kts[fb], pj, pj + 1)
                nc.sync.dma_start(xt[:, pj, :, :], xt_d[:, pj, :, :])
            for pj in range(NJ):
                for fb in range(RAMP):
                    emit_channel(fb, pj, kts[fb])
            for fb in range(RAMP):
                emit_combine(fb)

            # ---- steady state ----
            for fb in range(RAMP, FB):
                kt = new_kt(fb)
                dma_kt(fb, kt, 0, NA)
                dma_kt(fb, kt, NA, NJ)
                open_fb(fb)
                for pj in range(NJ):
                    emit_channel(fb, pj, kt)
                emit_combine(fb)

    nc.compile()
    _CACHE["nc"] = nc
    return nc


def _prep_inputs(x, kernels):
    G, BT, _, _, alpha, beta = _mats()
    Kt = np.einsum("ji,idf->jdf", G, kernels[::-1].astype(np.float64)) \
        * beta[:, None, None]
    Kt = Kt[J_ORDER]                      # device channel-position order
    kt16 = np.ascontiguousarray(
        Kt.reshape(NJ, KD, 128, FB, 128).transpose(3, 2, 0, 1, 4)
    ).astype(np.float16)
    need = M * (U - 1) + NJ               # 4099 padded rows
    idx = M * np.arange(U)
    in_maps = []
    for c in range(N_CORES):
        b, h = divmod(c, 2)
        xp = np.zeros((need, D), dtype=np.float64)
        s0 = h * T - (R - 1)
        lo = max(s0, 0)
        xp[lo - s0: need] = x[b, lo: s0 + need]
        Wn = np.stack([xp[idx + l] for l in range(NJ)])      # [11, U, D]
        Xt = np.einsum("jl,lud->jud", BT, Wn) * alpha[:, None, None]
        Xt = Xt[J_ORDER]                  # device channel-position order
        Xr = Xt.reshape(NJ, U, KD, 128).transpose(3, 0, 2, 1)
        in_maps.append({"kt": kt16,
                        "xt": np.ascontiguousarray(Xr).astype(np.float16)})
    return in_maps


def kernel(x, kernels, biases, trace=False):
    from concourse.bass_utils import run_bass_kernel_spmd

    x = np.asarray(x, dtype=np.float32)
    kernels = np.asarray(kernels, dtype=np.float32)
    biases = np.asarray(biases, dtype=np.float32)
    nc = _build()
    in_maps = _prep_inputs(x, kernels)
    res = run_bass_kernel_spmd(nc, in_maps, core_ids=list(range(N_CORES)),
                               trace=trace)
    out = np.empty((B, S, F), dtype=np.float32)
    for c in range(N_CORES):
        b, h = divmod(c, 2)
        o = res.results[c]["outT"]            # [FB, 128, M, U] fp16
        out[b, h * T:(h + 1) * T] = (
            o.transpose(3, 2, 0, 1).reshape(T, F).astype(np.float32))
    bias_total = biases.astype(np.float32).sum(axis=0)
    if np.any(bias_total):
        out += bias_total
    if trace:
        kernel.last_exec_time_ns = res.exec_time_ns
    return out


# revision 15
# speedup vs baseline: 1.3862x; 1.3862x over previous
"""AltConv via Winograd F(8,4) in fp16 on 8 TRN2 NeuronCores.

out[s] = sum_{i=0..3} K_i x[s-i].  Outputs in blocks of M=8 from NJ=11
Winograd-channel matmuls instead of 32 (2.9x fewer PE cycles than direct):

  w_l(u) = x[8u-3+l], l=0..10
  x~_j = alpha_j (BT w)_j   (host f64)    K~_j = beta_j (G Krev)_j (host f64)
  P_j  = x~_j @ K~_j        (device TensorE fp16, f32 PSUM accum over D)
  out[8u+t] = sum_j A[t,j]/(alpha_j beta_j) P_j(u)

Points {0, +-1, +-a, +-b, +-c, d, inf} (numerically optimized); per-channel
scales keep every fp16 plane in range; zero/inf channels scaled so their
combine coefficient is exactly 1 (plain adds).

Sharding: data-parallel over (batch, seq-half) -> 8 shards of T=4096 tokens,
U=512 output blocks each (one PSUM bank per P plane, rotating pool of 6).
Combine split across engines: ScalarE drains planes to fp16 + scales the
single-point channel per t; DVE does S/D pairs + 3 stt per output t;
GpSimd (parallel SBUF port) adds the pre-scaled single/zero/inf terms and
issues fp16 output stores.  First two feature blocks are channel-interleaved
so the PE keeps pace with the initial x~ stream.
"""

import numpy as np

B, S, D, F, R = 4, 8192, 1024, 1024, 4
N_CORES = 8
T = S // 2            # tokens per core
M = 8                 # outputs per Winograd block
NJ = M + R - 1        # 11 Winograd channels
KD = D // 128
FB = F // 128
U = T // M            # 512 blocks per core (exact)
RAMP = 2              # feature blocks interleaved during x~ stream-in

# optimized points: pairs (1, a, b, c), single d, zero, inf
PAIR_VALS = [1.0, 0.3744, 0.7256, 1.6749]
SINGLES = [4.0762]
NPAIR = len(PAIR_VALS)
J_SINGLE = 2 * NPAIR          # 8
J_ZERO = NJ - 2               # 9
J_INF = NJ - 1                # 10
J_ORDER = list(range(2 * NPAIR)) + [J_SINGLE, J_ZERO, J_INF]
_CACHE = {}


def _mats():
    pts = []
    for p in PAIR_VALS:
        pts += [p, -p]
    pts += list(SINGLES) + [0.0]
    n = NJ
    G = np.zeros((n, R))
    for j, p in enumerate(pts):
        G[j] = [p ** e for e in range(R)]
    G[-1, R - 1] = 1.0
    V = np.zeros((n, n))
    for j, p in enumerate(pts):
        V[j] = [p ** e for e in range(n)]
    V[-1, -1] = 1.0
    BT = np.linalg.inv(V).T
    A = np.zeros((M, n))
    for j, p in enumerate(pts):
        A[:, j] = [p ** t for t in range(M)]
    A[M - 1, n - 1] = 1.0
    alpha = 1.0 / np.linalg.norm(BT, axis=1)
    beta = 64.0 / np.linalg.norm(G, axis=1)
    for i in range(NPAIR):
        alpha[2 * i] = alpha[2 * i + 1] = min(alpha[2 * i], alpha[2 * i + 1])
        beta[2 * i] = beta[2 * i + 1] = min(beta[2 * i], beta[2 * i + 1])
    # chain base (pair 1) and the zero/inf add-terms need coefficient 1
    beta[0] = beta[1] = 1.0 / alpha[0]
    beta[J_ZERO] = 1.0 / alpha[J_ZERO]
    beta[J_INF] = 1.0 / alpha[J_INF]
    Ap = A / (alpha * beta)[None, :]
    assert abs(Ap[0, J_ZERO] - 1.0) < 1e-12
    assert abs(Ap[M - 1, J_INF] - 1.0) < 1e-12
    return G, BT, A, Ap, alpha, beta


def _build():
    if "nc" in _CACHE:
        return _CACHE["nc"]
    import concourse.tile as tile
    from concourse import bacc, mybir

    _, _, _, Ap, _, _ = _mats()
    nc = bacc.Bacc("TRN2", target_bir_lowering=False, debug=False,
                   num_devices=N_CORES)
    f16 = mybir.dt.float16
    f32 = mybir.dt.float32
    mult = mybir.AluOpType.mult
    add = mybir.AluOpType.add

    # channel storage on DRAM is permuted to processing order J_ORDER;
    # position pj holds math-channel J_ORDER[pj]
    NA = 6                       # channels in the first kt group tile
    xt_d = nc.dram_tensor("xt", [128, NJ, KD, U], f16, kind="ExternalInput")
    kt_d = nc.dram_tensor("kt", [FB, 128, NJ, KD, 128], f16,
                          kind="ExternalInput")
    out_d = nc.dram_tensor("outT", [FB, 128, M, U], f16,
                           kind="ExternalOutput")

    with tile.TileContext(nc) as tc:
        with (
            tc.tile_pool(name="xpool", bufs=1) as xpool,
            tc.tile_pool(name="kpool", bufs=1) as kpool,
            tc.tile_pool(name="psum", bufs=1, space="PSUM") as ppool,
            tc.tile_pool(name="stage", bufs=1) as spool,
        ):
            xt = xpool.tile([128, NJ, KD, U], f16)
            state = {}

            def new_kt(fb):
                # two channel-group tiles so buffer recycling (and thus the
                # next block's prefetch) happens at half-block granularity
                ka = kpool.tile([128, NA, KD, 128], f16, name=f"ktA{fb}",
                                tag="ktA", bufs=2)
                kb = kpool.tile([128, NJ - NA, KD, 128], f16,
                                name=f"ktB{fb}", tag="ktB", bufs=2)
                return (ka, kb)

            def emit_channel(fb, pj, kt):
                j = J_ORDER[pj]
                ktile = kt[0] if pj < NA else kt[1]
                lj = pj if pj < NA else pj - NA
                st, pc, sd, tmpd, tmpc = state[fb]
                P = ppool.tile([128, U], f32, tag="P", bufs=6,
                               name=f"P{fb}_{j}")
                for kd in range(KD):
                    nc.tensor.matmul(
                        P, ktile[:, lj, kd, :], xt[:, pj, kd, :],
                        start=(kd == 0), stop=(kd == KD - 1),
                    )
                if j == J_SINGLE:
                    # ScalarE scales the single-point plane per t, from PSUM
                    for t in range(M):
                        nc.scalar.mul(tmpd[:, t, :], P,
                                      float(Ap[t, J_SINGLE]))
                else:
                    tag = "pcp" if j < 2 * NPAIR else f"pc{j}"
                    bufs = 4 if j < 2 * NPAIR else 2
                    c = spool.tile([128, U], f16, tag=tag, bufs=bufs,
                                   name=f"pc{fb}_{j}")
                    nc.scalar.copy(c, P)
                    pc[j] = c
                    if j < 2 * NPAIR and j % 2 == 1:
                        i = j // 2
                        s_ = spool.tile([128, U], f16, tag=f"sd{i}S",
                                        bufs=2, name=f"S{fb}_{i}")
                        d_ = spool.tile([128, U], f16, tag=f"sd{i}D",
                                        bufs=2, name=f"D{fb}_{i}")
                        eng_s = nc.gpsimd if i in (0, 2) else nc.vector
                        eng_s.tensor_add(s_, pc[j - 1], pc[j])
                        nc.vector.tensor_sub(d_, pc[j - 1], pc[j])
                        sd[(i, 0)] = s_
                        sd[(i, 1)] = d_

            def open_fb(fb):
                st = spool.tile([128, M, U], f16, tag="st", bufs=2,
                                name=f"st{fb}")
                tmpd = spool.tile([128, M, U], f16, tag="tmpd", bufs=2,
                                  name=f"tmpd{fb}")
                tmpc = spool.tile([128, M, U], f16, tag="tmpc", bufs=2,
                                  name=f"tmpc{fb}")
                state[fb] = (st, {}, {}, tmpd, tmpc)

            def emit_combine(fb):
                st, pc, sd, tmpd, tmpc = state[fb]
                # ScalarE pre-scales the c-pair term per t (after all drains)
                for t in range(M):
                    nc.scalar.mul(tmpc[:, t, :], sd[(NPAIR - 1, t % 2)],
                                  float(Ap[t, 2 * (NPAIR - 1)]))
                for t in range(M):
                    acc = st[:, t, :]
                    par = t % 2
                    nc.vector.scalar_tensor_tensor(
                        acc, sd[(1, par)], float(Ap[t, 2]), sd[(0, par)],
                        mult, add)
                    nc.vector.scalar_tensor_tensor(
                        acc, sd[(2, par)], float(Ap[t, 4]), acc,
                        mult, add)
                    nc.vector.tensor_add(acc, tmpc[:, t, :], acc)
                    nc.vector.tensor_add(acc, tmpd[:, t, :], acc)
                    if t == 0:
                        nc.gpsimd.tensor_add(acc, pc[J_ZERO], acc)
                    if t == M - 1:
                        nc.gpsimd.tensor_add(acc, pc[J_INF], acc)
                nc.gpsimd.dma_start(out_d[fb, :, :, :], st[:, :, :])

            def dma_kt(fb, kt, pj_lo, pj_hi):
                for pj in range(pj_lo, pj_hi):
                    ktile = kt[0] if pj < NA else kt[1]
                    lj = pj if pj < NA else pj - NA
                    nc.sync.dma_start(ktile[:, lj, :, :],
                                      kt_d[fb, :, pj, :, :])

            # ---- ramp: first RAMP feature blocks channel-interleaved ----
            kts = [new_kt(fb) for fb in range(RAMP)]
            for fb in range(RAMP):
                open_fb(fb)
            for pj in range(NJ):
                for fb in range(RAMP):
                    dma_kt(fb, kts[fb], pj, pj + 1)
                nc.sync.dma_start(xt[:, pj, :, :], xt_d[:, pj, :, :])
            for pj in range(NJ):
                for fb in range(RAMP):
                    emit_channel(fb, pj, kts[fb])
            for fb in range(RAMP):
                emit_combine(fb)

            # ---- steady state ----
            for fb in range(RAMP, FB):
                kt = new_kt(fb)
                dma_kt(fb, kt, 0, NA)
                dma_kt(fb, kt, NA, NJ)
                open_fb(fb)
                for pj in range(NJ):
                    emit_channel(fb, pj, kt)
                emit_combine(fb)

    nc.compile()
    _CACHE["nc"] = nc
    return nc


def _prep_inputs(x, kernels):
    G, BT, _, _, alpha, beta = _mats()
    Kt = np.einsum("ji,idf->jdf", G, kernels[::-1].astype(np.float64)) \
        * beta[:, None, None]
    Kt = Kt[J_ORDER]                      # device channel-position order
    kt16 = np.ascontiguousarray(
        Kt.reshape(NJ, KD, 128, FB, 128).transpose(3, 2, 0, 1, 4)
    ).astype(np.float16)
    need = M * (U - 1) + NJ               # 4099 padded rows
    idx = M * np.arange(U)
    in_maps = []
    for c in range(N_CORES):
        b, h = divmod(c, 2)
        xp = np.zeros((need, D), dtype=np.float64)
        s0 = h * T - (R - 1)
        lo = max(s0, 0)
        xp[lo - s0: need] = x[b, lo: s0 + need]
        Wn = np.stack([xp[idx + l] for l in range(NJ)])      # [11, U, D]
        Xt = np.einsum("jl,lud->jud", BT, Wn) * alpha[:, None, None]
        Xt = Xt[J_ORDER]                  # device channel-position order
        Xr = Xt.reshape(NJ, U, KD, 128).transpose(3, 0, 2, 1)
        in_maps.append({"kt": kt16,
                        "xt": np.ascontiguousarray(Xr).astype(np.float16)})
    return in_maps


def kernel(x, kernels, biases, trace=False):
    from concourse.bass_utils import run_bass_kernel_spmd

    x = np.asarray(x, dtype=np.float32)
    kernels = np.asarray(kernels, dtype=np.float32)
    biases = np.asarray(biases, dtype=np.float32)
    nc = _build()
    in_maps = _prep_inputs(x, kernels)
    res = run_bass_kernel_spmd(nc, in_maps, core_ids=list(range(N_CORES)),
                               trace=trace)
    out = np.empty((B, S, F), dtype=np.float32)
    for c in range(N_CORES):
        b, h = divmod(c, 2)
        o = res.results[c]["outT"]            # [FB, 128, M, U] fp16
        out[b, h * T:(h + 1) * T] = (
            o.transpose(3, 2, 0, 1).reshape(T, F).astype(np.float32))
    bias_total = biases.astype(np.float32).sum(axis=0)
    if np.any(bias_total):
        out += bias_total
    if trace:
        kernel.last_exec_time_ns = res.exec_time_ns
    return out


# revision 16
# speedup vs baseline: 1.5438x; 1.1137x over previous
"""AltConv via Winograd F(8,4) in fp16 on 8 TRN2 NeuronCores.

out[s] = sum_{i=0..3} K_i x[s-i].  Outputs in blocks of M=8 from NJ=11
Winograd-channel matmuls instead of 32 (2.9x fewer PE cycles than direct):

  w_l(u) = x[8u-3+l], l=0..10
  x~_j = alpha_j (BT w)_j   (host f64)    K~_j = beta_j (G Krev)_j (host f64)
  P_j  = x~_j @ K~_j        (device TensorE fp16, f32 PSUM accum over D)
  out[8u+t] = sum_j A[t,j]/(alpha_j beta_j) P_j(u)

Points {0, +-1, +-a, +-b, +-c, d, inf} (numerically optimized); per-channel
scales keep every fp16 plane in range; zero/inf channels scaled so their
combine coefficient is exactly 1 (plain adds).

Sharding: data-parallel over (batch, seq-half) -> 8 shards of T=4096 tokens,
U=512 output blocks each (one PSUM bank per P plane, rotating pool of 6).
Combine split across engines: ScalarE drains planes to fp16 + scales the
single-point channel per t; DVE does S/D pairs + 3 stt per output t;
GpSimd (parallel SBUF port) adds the pre-scaled single/zero/inf terms and
issues fp16 output stores.  First two feature blocks are channel-interleaved
so the PE keeps pace with the initial x~ stream.
"""

import numpy as np

B, S, D, F, R = 4, 8192, 1024, 1024, 4
N_CORES = 8
T = S // 2            # tokens per core
M = 8                 # outputs per Winograd block
NJ = M + R - 1        # 11 Winograd channels
KD = D // 128
FB = F // 128
U = T // M            # 512 blocks per core (exact)
RAMP = 2              # feature blocks interleaved during x~ stream-in

# optimized points: pairs (1, a, b, c), single d, zero, inf
PAIR_VALS = [1.0, 0.3744, 0.7256, 1.6749]
SINGLES = [4.0762]
NPAIR = len(PAIR_VALS)
J_SINGLE = 2 * NPAIR          # 8
J_ZERO = NJ - 2               # 9
J_INF = NJ - 1                # 10
J_ORDER = [J_SINGLE] + list(range(2 * NPAIR)) + [J_ZERO, J_INF]
_CACHE = {}


def _mats():
    pts = []
    for p in PAIR_VALS:
        pts += [p, -p]
    pts += list(SINGLES) + [0.0]
    n = NJ
    G = np.zeros((n, R))
    for j, p in enumerate(pts):
        G[j] = [p ** e for e in range(R)]
    G[-1, R - 1] = 1.0
    V = np.zeros((n, n))
    for j, p in enumerate(pts):
        V[j] = [p ** e for e in range(n)]
    V[-1, -1] = 1.0
    BT = np.linalg.inv(V).T
    A = np.zeros((M, n))
    for j, p in enumerate(pts):
        A[:, j] = [p ** t for t in range(M)]
    A[M - 1, n - 1] = 1.0
    alpha = 1.0 / np.linalg.norm(BT, axis=1)
    beta = 64.0 / np.linalg.norm(G, axis=1)
    for i in range(NPAIR):
        alpha[2 * i] = alpha[2 * i + 1] = min(alpha[2 * i], alpha[2 * i + 1])
        beta[2 * i] = beta[2 * i + 1] = min(beta[2 * i], beta[2 * i + 1])
    # chain base (pair 1) and the zero/inf add-terms need coefficient 1
    beta[0] = beta[1] = 1.0 / alpha[0]
    beta[J_ZERO] = 1.0 / alpha[J_ZERO]
    beta[J_INF] = 1.0 / alpha[J_INF]
    Ap = A / (alpha * beta)[None, :]
    assert abs(Ap[0, J_ZERO] - 1.0) < 1e-12
    assert abs(Ap[M - 1, J_INF] - 1.0) < 1e-12
    return G, BT, A, Ap, alpha, beta


def _build():
    if "nc" in _CACHE:
        return _CACHE["nc"]
    import concourse.tile as tile
    from concourse import bacc, mybir

    _, _, _, Ap, _, _ = _mats()
    nc = bacc.Bacc("TRN2", target_bir_lowering=False, debug=False,
                   num_devices=N_CORES)
    f16 = mybir.dt.float16
    f32 = mybir.dt.float32
    mult = mybir.AluOpType.mult
    add = mybir.AluOpType.add

    # channel storage on DRAM is permuted to processing order J_ORDER;
    # position pj holds math-channel J_ORDER[pj]
    NA = 6                       # channels in the first kt group tile
    xt_d = nc.dram_tensor("xt", [128, NJ, KD, U], f16, kind="ExternalInput")
    kt_d = nc.dram_tensor("kt", [FB, 128, NJ, KD, 128], f16,
                          kind="ExternalInput")
    out_d = nc.dram_tensor("outT", [FB, 128, M, U], f16,
                           kind="ExternalOutput")

    with tile.TileContext(nc) as tc:
        with (
            tc.tile_pool(name="xpool", bufs=1) as xpool,
            tc.tile_pool(name="kpool", bufs=1) as kpool,
            tc.tile_pool(name="psum", bufs=1, space="PSUM") as ppool,
            tc.tile_pool(name="stage", bufs=1) as spool,
        ):
            xt = xpool.tile([128, NJ, KD, U], f16)
            state = {}

            def new_kt(fb):
                # two channel-group tiles so buffer recycling (and thus the
                # next block's prefetch) happens at half-block granularity
                ka = kpool.tile([128, NA, KD, 128], f16, name=f"ktA{fb}",
                                tag="ktA", bufs=2)
                kb = kpool.tile([128, NJ - NA, KD, 128], f16,
                                name=f"ktB{fb}", tag="ktB", bufs=2)
                return (ka, kb)

            def emit_channel(fb, pj, kt):
                j = J_ORDER[pj]
                ktile = kt[0] if pj < NA else kt[1]
                lj = pj if pj < NA else pj - NA
                st, pc, sd, tmpd, tmpc = state[fb]
                P = ppool.tile([128, U], f32, tag="P", bufs=8,
                               name=f"P{fb}_{j}")
                for kd in range(KD):
                    nc.tensor.matmul(
                        P, ktile[:, lj, kd, :], xt[:, pj, kd, :],
                        start=(kd == 0), stop=(kd == KD - 1),
                    )
                if j == J_SINGLE:
                    # ScalarE scales the single-point plane per t, from PSUM
                    for t in range(M):
                        nc.scalar.mul(tmpd[:, t, :], P,
                                      float(Ap[t, J_SINGLE]))
                else:
                    tag = "pcp" if j < 2 * NPAIR else f"pc{j}"
                    bufs = 4 if j < 2 * NPAIR else 2
                    c = spool.tile([128, U], f16, tag=tag, bufs=bufs,
                                   name=f"pc{fb}_{j}")
                    nc.scalar.copy(c, P)
                    pc[j] = c
                    if j < 2 * NPAIR and j % 2 == 1:
                        i = j // 2
                        s_ = spool.tile([128, U], f16, tag=f"sd{i}S",
                                        bufs=2, name=f"S{fb}_{i}")
                        d_ = spool.tile([128, U], f16, tag=f"sd{i}D",
                                        bufs=2, name=f"D{fb}_{i}")
                        nc.vector.tensor_add(s_, pc[j - 1], pc[j])
                        nc.vector.tensor_sub(d_, pc[j - 1], pc[j])
                        sd[(i, 0)] = s_
                        sd[(i, 1)] = d_

            def open_fb(fb):
                st = spool.tile([128, M, U], f16, tag="st", bufs=2,
                                name=f"st{fb}")
                tmpd = spool.tile([128, M, U], f16, tag="tmpd", bufs=2,
                                  name=f"tmpd{fb}")
                tmpc = spool.tile([128, M, U], f16, tag="tmpc", bufs=2,
                                  name=f"tmpc{fb}")
                state[fb] = (st, {}, {}, tmpd, tmpc)

            def emit_combine(fb):
                st, pc, sd, tmpd, tmpc = state[fb]
                # ScalarE pre-scales the c-pair term per t (after all drains)
                for t in range(M):
                    nc.scalar.mul(tmpc[:, t, :], sd[(NPAIR - 1, t % 2)],
                                  float(Ap[t, 2 * (NPAIR - 1)]))
                for t in range(M):
                    acc = st[:, t, :]
                    par = t % 2
                    nc.vector.scalar_tensor_tensor(
                        acc, sd[(1, par)], float(Ap[t, 2]), sd[(0, par)],
                        mult, add)
                    nc.vector.scalar_tensor_tensor(
                        acc, sd[(2, par)], float(Ap[t, 4]), acc,
                        mult, add)
                    nc.vector.tensor_add(acc, tmpc[:, t, :], acc)
                    nc.vector.tensor_add(acc, tmpd[:, t, :], acc)
                    if t == 0:
                        nc.vector.tensor_add(acc, pc[J_ZERO], acc)
                    if t == M - 1:
                        nc.vector.tensor_add(acc, pc[J_INF], acc)
                nc.gpsimd.dma_start(out_d[fb, :, :, :], st[:, :, :])

            def dma_kt(fb, kt, pj_lo, pj_hi):
                for pj in range(pj_lo, pj_hi):
                    ktile = kt[0] if pj < NA else kt[1]
                    lj = pj if pj < NA else pj - NA
                    nc.sync.dma_start(ktile[:, lj, :, :],
                                      kt_d[fb, :, pj, :, :])

            # ---- ramp: first RAMP feature blocks channel-interleaved ----
            kts = [new_kt(fb) for fb in range(RAMP)]
            for fb in range(RAMP):
                open_fb(fb)
            for pj in range(NJ):
                for fb in range(RAMP):
                    dma_kt(fb, kts[fb], pj, pj + 1)
                nc.sync.dma_start(xt[:, pj, :, :], xt_d[:, pj, :, :])
            for pj in range(NJ):
                for fb in range(RAMP):
                    emit_channel(fb, pj, kts[fb])
            for fb in range(RAMP):
                emit_combine(fb)

            # ---- steady state ----
            for fb in range(RAMP, FB):
                kt = new_kt(fb)
                dma_kt(fb, kt, 0, NA)
                dma_kt(fb, kt, NA, NJ)
                open_fb(fb)
                for pj in range(NJ):
                    emit_channel(fb, pj, kt)
                emit_combine(fb)

    nc.compile()
    _CACHE["nc"] = nc
    return nc


def _prep_inputs(x, kernels):
    G, BT, _, _, alpha, beta = _mats()
    Kt = np.einsum("ji,idf->jdf", G, kernels[::-1].astype(np.float64)) \
        * beta[:, None, None]
    Kt = Kt[J_ORDER]                      # device channel-position order
    kt16 = np.ascontiguousarray(
        Kt.reshape(NJ, KD, 128, FB, 128).transpose(3, 2, 0, 1, 4)
    ).astype(np.float16)
    need = M * (U - 1) + NJ               # 4099 padded rows
    idx = M * np.arange(U)
    in_maps = []
    for c in range(N_CORES):
        b, h = divmod(c, 2)
        xp = np.zeros((need, D), dtype=np.float64)
        s0 = h * T - (R - 1)
        lo = max(s0, 0)
        xp[lo - s0: need] = x[b, lo: s0 + need]
        Wn = np.stack([xp[idx + l] for l in range(NJ)])      # [11, U, D]
        Xt = np.einsum("jl,lud->jud", BT, Wn) * alpha[:, None, None]
        Xt = Xt[J_ORDER]                  # device channel-position order
        Xr = Xt.reshape(NJ, U, KD, 128).transpose(3, 0, 2, 1)
        in_maps.append({"kt": kt16,
                        "xt": np.ascontiguousarray(Xr).astype(np.float16)})
    return in_maps


def kernel(x, kernels, biases, trace=False):
    from concourse.bass_utils import run_bass_kernel_spmd

    x = np.asarray(x, dtype=np.float32)
    kernels = np.asarray(kernels, dtype=np.float32)
    biases = np.asarray(biases, dtype=np.float32)
    nc = _build()
    in_maps = _prep_inputs(x, kernels)
    res = run_bass_kernel_spmd(nc, in_maps, core_ids=list(range(N_CORES)),
                               trace=trace)
    out = np.empty((B, S, F), dtype=np.float32)
    for c in range(N_CORES):
        b, h = divmod(c, 2)
        o = res.results[c]["outT"]            # [FB, 128, M, U] fp16
        out[b, h * T:(h + 1) * T] = (
            o.transpose(3, 2, 0, 1).reshape(T, F).astype(np.float32))
    bias_total = biases.astype(np.float32).sum(axis=0)
    if np.any(bias_total):
        out += bias_total
    if trace:
        kernel.last_exec_time_ns = res.exec_time_ns
    return out
